# revision 1
# baseline (speedup 1.0000x reference)
"""LinearRNN final-state kernel for 8 Trainium2 NeuronCores.

Reference computation:
    u_t = Wxh @ x_t + bxh            (input projection)
    h_t = u_t + Whh @ h_{t-1}        (recurrence over T=1024 steps)
    return h_T                        -> [B=32, H=512]

The recurrence is linear, so the final state is
    h_T = sum_t u_t @ A^(T-1-t),  A = Whh^T   (row-vector convention).
Instead of a sequential scan we use a binary tree fold: at level l adjacent
sequence elements pair up as  v' = v_odd + v_even @ A^(2^l).  Ten levels
collapse T=1024 -> 1.  All the work becomes dense GEMMs; the only sequential
chain is 9 squarings  A^(2^(l+1)) = A^(2^l) @ A^(2^l).

Sharding: data-parallel over batch (B=32 -> 4 rows/core on 8 cores);
weights and the squaring chain are replicated.

On-chip layout: sequence data lives transposed, [H, seq-cols], H on
partitions in 4 chunks of 128, so the level matrices are the stationary
matmul operand and the sequence streams through the PE array.
"""

import numpy as np

B, T, IN, H = 32, 1024, 256, 512
NCORES = 8
BC = B // NCORES          # 4 batch rows per core
COLS = BC * T             # 4096 sequence columns per core
HC = H // 128             # 4 hidden-dim chunks of 128
ICH = IN // 128           # 2 input-dim chunks
NLVL = 10                 # log2(T)

MM_DTYPE = "f32r"         # "f32": exact 4-pass fp32 | "f32r": fast fp32

_cache: dict = {}


def _build():
    import concourse.bass as bass
    import concourse.mybir as mybir
    from concourse import bacc
    from concourse.tile import TileContext
    from concourse.masks import make_identity

    f32 = mybir.dt.float32
    mmdt = {"f32": f32, "f32r": mybir.dt.float32r}[MM_DTYPE]

    def mc_(ap):
        return ap

    nc = bacc.Bacc(None)
    x_d = nc.declare_dram_parameter("x", [COLS, IN], f32, isOutput=False)
    wxh_d = nc.declare_dram_parameter("Wxh", [H, IN], f32, isOutput=False)
    bxh_d = nc.declare_dram_parameter("bxh", [H], f32, isOutput=False)
    whh_d = nc.declare_dram_parameter("Whh", [H, H], f32, isOutput=False)
    # Output stays in on-chip layout [128, HC*BC]; host unscrambles.
    out_d = nc.declare_dram_parameter("h_out", [128, HC * BC], f32, isOutput=True)

    ACT_COPY = mybir.ActivationFunctionType.Copy
    ACT_IDENT = mybir.ActivationFunctionType.Identity

    with TileContext(nc) as tc:
        with (
            tc.tile_pool(name="const", bufs=1) as cpool,
            tc.tile_pool(name="lvl", bufs=1) as lpool,
            tc.tile_pool(name="stream", bufs=4) as xpool,
            tc.tile_pool(name="mats", bufs=3) as spool,
            tc.tile_pool(name="mm", bufs=4, space="PSUM") as mmpool,
            tc.tile_pool(name="tr", bufs=4, space="PSUM") as trpool,
        ):
            ident = cpool.tile([128, 128], f32, tag="ident")
            make_identity(nc, ident[:])

            # PE warm-up: dummy matmuls on the identity keep the PE busy
            # through the initial weight-DMA wait and complete the clock
            # ramp before real work arrives.
            warm = mmpool.tile([128, 128], f32, tag="mm")
            for _ in range(8):
                nc.tensor.matmul(warm[:], ident[:], ident[:], start=True, stop=True)

            if mmdt != f32:
                ident_r = cpool.tile([128, 128], mmdt, tag="identr")
                nc.vector.tensor_copy(ident_r[:], ident[:])
            else:
                ident_r = ident

            # Weights go on the ACT HWDGE ring so the x-group loads on the
            # SP ring are not queued behind them.  Wxh first: the first PE
            # work (WxhT transposes) depends on it.
            wxh_t = cpool.tile([128, HC, IN], f32, tag="wxh")
            nc.sync.dma_start(wxh_t[:], wxh_d.rearrange("(c p) f -> p c f", p=128))
            xg_pre = []
            for g in range(2):
                xg = xpool.tile([128, 4, IN], f32, tag="xg")
                nc.sync.dma_start(
                    xg[:],
                    x_d[g * 512:(g + 1) * 512, :].rearrange(
                        "(j p) i -> p j i", p=128
                    ),
                )
                xg_pre.append(xg)
            w_t = cpool.tile([128, HC, H], f32, tag="whh")
            nc.scalar.dma_start(w_t[:], whh_d.rearrange("(c p) f -> p c f", p=128))
            bias = cpool.tile([128, HC], f32, tag="bias")
            nc.scalar.dma_start(bias[:], bxh_d.rearrange("(c p) -> p c", p=128))

            def transpose_quad(dst_ap, srcs, copy_engine="dve"):
                """Transpose four [128,128] blocks into one PSUM bank, then
                move them to SBUF with a single wide copy.  When every source
                is f32r-produced the transpose runs in f32r (1.5 vs 2
                cycles/row on the PE)."""
                all_r = mmdt != f32 and all(s.dtype == mmdt for s in srcs)
                idn = ident_r if all_r else ident
                tp = trpool.tile(
                    [128, 128 * len(srcs)], mmdt if all_r else f32, tag="tp"
                )
                for i, s in enumerate(srcs):
                    if not all_r and s.dtype != f32:
                        s = s.bitcast(f32)
                    nc.tensor.transpose(tp[:, i * 128:(i + 1) * 128], s, idn[:])
                if copy_engine == "act":
                    nc.scalar.activation(dst_ap, tp[:], ACT_COPY)
                else:
                    nc.vector.tensor_copy(dst_ap, tp[:])

            # WxhT[p, ic, f] = Wxh[f, ic*128+p]  (lhsT for the projection)
            wxhT = cpool.tile([128, ICH, H], mmdt, tag="wxhT")
            for ic in range(ICH):
                transpose_quad(
                    wxhT[:, ic, :],
                    [wxh_t[:, rc, ic * 128:(ic + 1) * 128] for rc in range(HC)],
                )

            # rounded copy of Whh for use as a matmul operand (ST_0)
            if mmdt != f32:
                w_r = spool.tile([128, HC, H], mmdt, tag="wr", bufs=1)
                for c in range(HC):
                    nc.vector.tensor_copy(w_r[:, c, :], w_t[:, c, :])
            else:
                w_r = w_t

            # S_0[p, kc, f] = A[kc*128+p, f] = Whh[f, kc*128+p]
            S = spool.tile([128, HC, H], mmdt, tag="S")
            for cc in range(HC):
                transpose_quad(
                    S[:, cc, :],
                    [w_t[:, rc, cc * 128:(cc + 1) * 128] for rc in range(HC)],
                )

            # ---- projection fused with tree level 0.
            # out_c = u_{2c+1} + u_{2c} A
            #       = x_{2c+1} Wxh^T + x_{2c} (Wxh^T A) + b(I + A)
            # Precompute G = Wxh^T A and b2 = b + bA once, then each column
            # pair costs 4 matmuls of N=256 and a single biased epilogue.
            wxh_r = spool.tile([128, HC, IN], mmdt, tag="wxr", bufs=1)
            for c in range(HC):
                nc.vector.tensor_copy(wxh_r[:, c, :], wxh_t[:, c, :])
            G = cpool.tile([128, ICH, H], mmdt, tag="G")
            for gc in range(ICH):
                ps = mmpool.tile([128, H], f32, tag="mm")
                for kc in range(HC):
                    nc.tensor.matmul(
                        ps[:],
                        wxh_r[:, kc, gc * 128:(gc + 1) * 128],
                        S[:, kc, :],
                        start=(kc == 0),
                        stop=(kc == HC - 1),
                    )
                nc.vector.tensor_copy(G[:, gc, :], ps[:])
            bias2 = cpool.tile([128, HC], f32, tag="bias2")
            for mcc in range(HC):
                ps = mmpool.tile([128, 1], f32, tag="mm")
                for kc in range(HC):
                    nc.tensor.matmul(
                        ps[:],
                        S[:, kc, mcc * 128:(mcc + 1) * 128].bitcast(f32),
                        bias[:, kc:kc + 1],
                        start=(kc == 0),
                        stop=(kc == HC - 1),
                    )
                nc.vector.tensor_add(
                    bias2[:, mcc:mcc + 1], ps[:], bias[:, mcc:mcc + 1]
                )

            NG = COLS // 512  # 8
            buf = lpool.tile([128, HC, COLS // 2], mmdt, tag="L1")
            for g in range(NG):
                if g < 2:
                    xg = xg_pre[g]
                else:
                    xg = xpool.tile([128, 4, IN], f32, tag="xg")
                    nc.sync.dma_start(
                        xg[:],
                        x_d[g * 512:(g + 1) * 512, :].rearrange(
                            "(j p) i -> p j i", p=128
                        ),
                    )
                # xT[p, ic, c] = x[512g + c, ic*128+p]
                xT = xpool.tile([128, ICH, 512], mmdt, tag="xT")
                for ic in range(ICH):
                    transpose_quad(
                        xT[:, ic, :],
                        [xg[:, j, ic * 128:(ic + 1) * 128] for j in range(4)],
                    )
                for mcc in range(HC):
                    ps = mmpool.tile([128, 256], f32, tag="mm")
                    for ic in range(ICH):
                        nc.tensor.matmul(
                            ps[:],
                            wxhT[:, ic, mcc * 128:(mcc + 1) * 128],
                            xT[:, ic, 1::2],
                            start=(ic == 0),
                            stop=False,
                        )
                    for ic in range(ICH):
                        nc.tensor.matmul(
                            ps[:],
                            G[:, ic, mcc * 128:(mcc + 1) * 128],
                            xT[:, ic, 0::2],
                            start=False,
                            stop=(ic == ICH - 1),
                        )
                    nc.scalar.activation(
                        buf[:, mcc, g * 256:(g + 1) * 256],
                        ps[:],
                        ACT_IDENT,
                        bias=bias2[:, mcc:mcc + 1],
                    )

            # ---- tree levels 1..9 with the squaring chain interleaved.
            # Emission order per step: (a) transposes for the next squaring,
            # (b) the PREVIOUS tree level (fills the PE while the transpose
            # and squaring epilogue copies land), (c) the squaring matmuls.
            S_of = {0: S}

            def emit_tree(lvl, buf):
                Sl = S_of[lvl]
                in_cols = COLS // (2 ** lvl)
                o_cols = in_cols // 2
                nbuf = lpool.tile([128, HC, o_cols], mmdt, tag=f"L{lvl + 1}")
                nwin = (o_cols + 511) // 512
                # At 2n == 256 a stride-1 f32r matmul over all columns (half
                # the cycles of a 128-wide strided one) beats discarding:
                # junk odd-column products are skipped by a strided PSUM read.
                dense = (o_cols == 128)
                if o_cols <= 64:
                    # Small level: all four H-chunks share one PSUM bank and
                    # a single wide epilogue add (cuts serial DVE latency).
                    n = o_cols
                    ps = mmpool.tile([128, HC, n], f32, tag="mm")
                    for mcc in range(HC):
                        for kc in range(HC):
                            nc.tensor.matmul(
                                ps[:, mcc, :],
                                mc_(Sl[:, kc, mcc * 128:(mcc + 1) * 128]),
                                mc_(buf[:, kc, 0:2 * n:2]),
                                start=(kc == 0),
                                stop=(kc == HC - 1),
                            )
                    nc.vector.tensor_add(
                        nbuf[:, :, :], ps[:], buf[:, :, 1:2 * n:2]
                    )
                    return nbuf
                for w in range(nwin):
                    n = min(512, o_cols - w * 512)
                    base = 1024 * w
                    for mcc in range(HC):
                        if dense:
                            ps = mmpool.tile([128, 2 * n], f32, tag="mm")
                            rd = ps[:, 0::2]
                        else:
                            ps = mmpool.tile([128, n], f32, tag="mm")
                            rd = ps[:]
                        for kc in range(HC):
                            rhs = (
                                buf[:, kc, base:base + 2 * n]
                                if dense
                                else buf[:, kc, base:base + 2 * n:2]
                            )
                            nc.tensor.matmul(
                                ps[:],
                                mc_(Sl[:, kc, mcc * 128:(mcc + 1) * 128]),
                                mc_(rhs),
                                start=(kc == 0),
                                stop=(kc == HC - 1),
                            )
                        nc.vector.tensor_add(
                            nbuf[:, mcc, w * 512:w * 512 + n],
                            rd,
                            buf[:, mcc, base + 1:base + 2 * n:2],
                        )
                return nbuf

            for lvl in range(1, NLVL - 2):
                # (a) transposes of S_{lvl-1} for the squaring
                if lvl == 1:
                    STl = w_r  # (A^1)^T = Whh natural, rounded for matmul
                else:
                    STl = spool.tile([128, HC, H], mmdt, tag="ST")
                    for jc in range(HC):
                        transpose_quad(
                            STl[:, jc, :],
                            [
                                S_of[lvl - 1][:, fc, jc * 128:(jc + 1) * 128]
                                for fc in range(HC)
                            ],
                            copy_engine="act" if jc % 2 else "dve",
                        )
                # (b) previous tree level: PE filler while copies land
                if lvl >= 2:
                    buf = emit_tree(lvl - 1, buf)
                # (c) squaring matmuls -> S_lvl
                Sp = S_of[lvl - 1]
                Snew = spool.tile([128, HC, H], mmdt, tag="S")
                for mcc in range(HC):
                    ps = mmpool.tile([128, 512], f32, tag="mm")
                    for jc in range(HC):
                        nc.tensor.matmul(
                            ps[:],
                            mc_(STl[:, jc, mcc * 128:(mcc + 1) * 128]),
                            mc_(Sp[:, jc, :]),
                            start=(jc == 0),
                            stop=(jc == HC - 1),
                        )
                    if mcc % 2:
                        nc.scalar.activation(Snew[:, mcc, :], ps[:], ACT_COPY)
                    else:
                        nc.vector.tensor_copy(Snew[:, mcc, :], ps[:])
                S_of[lvl] = Snew

            buf = emit_tree(NLVL - 3, buf)

            # Levels 8 and 9 without materializing A^256 / A^512:
            # apply S7 = A^128 repeatedly (2x for level 8, 4x for level 9).
            S7 = S_of[NLVL - 3]

            def apply_chain(buf, n_out, k_apps, name):
                """v' = v_odd + v_even @ S7^k_apps, n_out output columns."""
                cur = None  # None means "read evens of buf"
                for a in range(k_apps):
                    ps = mmpool.tile([128, HC, n_out], f32, tag="mm")
                    for mcc in range(HC):
                        for kc in range(HC):
                            rhs = (
                                buf[:, kc, 0:2 * n_out:2]
                                if cur is None
                                else cur[:, kc, :]
                            )
                            nc.tensor.matmul(
                                ps[:, mcc, :],
                                mc_(S7[:, kc, mcc * 128:(mcc + 1) * 128]),
                                mc_(rhs),
                                start=(kc == 0),
                                stop=(kc == HC - 1),
                            )
                    if a < k_apps - 1:
                        cur = lpool.tile(
                            [128, HC, n_out], mmdt, tag=f"{name}s{a}"
                        )
                        nc.vector.tensor_copy(cur[:, :, :], ps[:])
                    else:
                        nbuf = lpool.tile([128, HC, n_out], mmdt, tag=name)
                        nc.vector.tensor_add(
                            nbuf[:, :, :], ps[:], buf[:, :, 1:2 * n_out:2]
                        )
                return nbuf

            buf = apply_chain(buf, 2 * BC, 2, "L9")   # level 8: A^256
            buf = apply_chain(buf, BC, 4, "L10")      # level 9: A^512
            

            # buf is now [128, HC, BC]: buf[p, c, b] = h_b[c*128+p].
            # Store in on-chip layout: one fully contiguous DMA.
            nc.sync.dma_start(
                out_d.rearrange("p (c b) -> p c b", b=BC),
                buf[:, :, :].bitcast(f32),
            )

    nc.compile()
    return nc


def _get_nc():
    if "nc" not in _cache:
        _cache["nc"] = _build()
    return _cache["nc"]


def _in_maps(inputs):
    x = np.ascontiguousarray(np.asarray(inputs["x"], dtype=np.float32))
    wxh = np.ascontiguousarray(np.asarray(inputs["Wxh"], dtype=np.float32))
    bxh = np.ascontiguousarray(np.asarray(inputs["bxh"], dtype=np.float32))
    whh = np.ascontiguousarray(np.asarray(inputs["Whh"], dtype=np.float32))
    return [
        dict(
            x=np.ascontiguousarray(
                x[c * BC:(c + 1) * BC].reshape(COLS, IN)
            ),
            Wxh=wxh,
            bxh=bxh,
            Whh=whh,
        )
        for c in range(NCORES)
    ]


def kernel(**inputs) -> np.ndarray:
    from concourse.bass_utils import run_bass_kernel_spmd

    res = run_bass_kernel_spmd(
        _get_nc(), _in_maps(inputs), list(range(NCORES))
    ).results
    return _assemble(res)


def _assemble(results) -> np.ndarray:
    outs = []
    for c in range(NCORES):
        o = np.asarray(results[c]["h_out"])      # [128, HC*BC] on-chip layout
        o = o.reshape(128, HC, BC).transpose(2, 1, 0).reshape(BC, H)
        outs.append(o)
    return np.concatenate(outs, axis=0).astype(np.float32)



# revision 14
# speedup vs baseline: 1.1999x; 1.1999x over previous
"""LinearRNN final-state kernel for 8 Trainium2 NeuronCores.

Reference computation:
    u_t = Wxh @ x_t + bxh            (input projection)
    h_t = u_t + Whh @ h_{t-1}        (recurrence over T=1024 steps)
    return h_T                        -> [B=32, H=512]

The recurrence is linear, so the final state is
    h_T = sum_t u_t @ A^(T-1-t),  A = Whh^T   (row-vector convention).
Tree fold: at level l adjacent sequence chunks pair up as
v' = v_odd + v_even @ A^(2^l).  All work is dense GEMMs; the only
sequential chain is the squaring chain A^(2^(l+1)) = (A^(2^l))^2.

Sharding: data-parallel over batch (B=32 -> 4 rows/core on 8 cores);
weights and the squaring chain are replicated.

This version:
  * all matmul operands are bf16 (1 cycle/row at any width on the PE),
  * every static transpose (x, Wxh, Whh) is done on the HOST as part of
    input marshalling -- the device receives matmul-ready layouts,
  * the squaring-chain transposes run on the DMA xbar
    (dma_start_transpose), not the PE,
  * the projection folds FOUR consecutive time steps at once using
    G_k = Wxh^T A^k for k=0..3, so the tree starts at level 2,
  * the last two tree levels run as a Horner chain with Q = A^256
    applied as two A^128 products, the odd-column adds folded into the
    PE via identity-matmul accumulation.
"""

import numpy as np
import ml_dtypes

B, T, IN, H = 32, 1024, 256, 512
NCORES = 8
BC = B // NCORES          # 4 batch rows per core
COLS = BC * T             # 4096 sequence columns per core
HC = H // 128             # 4 hidden-dim chunks of 128
ICH = IN // 128           # 2 input-dim chunks
NG = 8                    # x groups of 512 columns
NSQ = 7                   # materialize A^2 .. A^128
WARMUPS = 20

_cache: dict = {}


def _build():
    import concourse.mybir as mybir
    from concourse import bacc
    from concourse.tile import TileContext

    f32 = mybir.dt.float32
    bf16 = mybir.dt.bfloat16

    nc = bacc.Bacc(None)
    xT_d = nc.declare_dram_parameter("xT", [128, ICH, COLS], bf16, isOutput=False)
    wxhT_d = nc.declare_dram_parameter("wxhT", [128, ICH, H], bf16, isOutput=False)
    wnat_d = nc.declare_dram_parameter("wnat", [128, HC, IN], bf16, isOutput=False)
    s0_d = nc.declare_dram_parameter("s0", [128, HC, H], bf16, isOutput=False)
    st1_d = nc.declare_dram_parameter("st1", [128, HC, H], bf16, isOutput=False)
    bias4_d = nc.declare_dram_parameter("bias4", [128, HC], f32, isOutput=False)
    id_d = nc.declare_dram_parameter("ident", [128, 128], bf16, isOutput=False)
    out_d = nc.declare_dram_parameter("h_out", [128, HC * BC], f32, isOutput=True)

    ACT_COPY = mybir.ActivationFunctionType.Copy
    ACT_IDENT = mybir.ActivationFunctionType.Identity

    with TileContext(nc) as tc:
        with (
            tc.tile_pool(name="const", bufs=1) as cpool,
            tc.tile_pool(name="lvl", bufs=1) as lpool,
            tc.tile_pool(name="stream", bufs=3) as xpool,
            tc.tile_pool(name="mats", bufs=1) as spool,
            tc.tile_pool(name="stm", bufs=2) as stpool,
            tc.tile_pool(name="mm", bufs=4, space="PSUM") as mmpool,
            tc.tile_pool(name="tr", bufs=2, space="PSUM") as trpool,
            tc.tile_pool(name="sm", bufs=2, space="PSUM") as wpool,
        ):
            # ---- constant loads, spread across the three HWDGE rings.
            # The DMA device serializes transfers in arrival order, so the
            # squaring chain's operands (st1, s0) go first.
            st1 = cpool.tile([128, HC, H], bf16, tag="st1")
            nc.sync.dma_start(st1[:], st1_d[:, :, :])
            s0 = cpool.tile([128, HC, H], bf16, tag="s0")
            nc.scalar.dma_start(s0[:], s0_d[:, :, :])
            wnat = cpool.tile([128, HC, IN], bf16, tag="wnat")
            nc.scalar.dma_start(wnat[:], wnat_d[:, :, :])
            wxhT = cpool.tile([128, ICH, H], bf16, tag="wxhT")
            nc.scalar.dma_start(wxhT[:], wxhT_d[:, :, :])
            ident = cpool.tile([128, 128], bf16, tag="ident")
            nc.sync.dma_start(ident[:], id_d[:, :])
            bias4 = cpool.tile([128, HC], f32, tag="bias4")
            nc.scalar.dma_start(bias4[:], bias4_d[:, :])

            # x halves (SP ring): [128, ICH, 2048] each
            xh_tiles = []
            for hf in range(2):
                xh = xpool.tile([128, ICH, 2048], bf16, tag="xh")
                nc.sync.dma_start(
                    xh[:], xT_d[:, :, hf * 2048:(hf + 1) * 2048]
                )
                xh_tiles.append(xh)

            # ---- PE warm-up on a memset tile (no DMA dependency): covers
            # the constant-DMA wait and starts the p-state ramp.
            warm_in = cpool.tile([128, 128], bf16, tag="warmin")
            nc.vector.memset(warm_in[:], 0.0)
            for _ in range(WARMUPS):
                warm = mmpool.tile([128, 128], f32, tag="mm")
                nc.tensor.matmul(
                    warm[:], warm_in[:], warm_in[:], start=True, stop=True
                )

            # ---- squaring-chain pieces ------------------------------
            S_of = {0: s0}
            ST_of = {1: st1}

            def emit_sq_transposes(lvl):
                """ST_{lvl}[p, jc, mcc*128+m] = P[mcc*128+m, jc*128+p] via
                PE transposes (the DMA xbar corrupts under concurrent
                multi-core use)."""
                Sp = S_of[lvl - 1]
                STl = stpool.tile([128, HC, H], bf16, tag="ST")
                for jc in range(HC):
                    tp = trpool.tile([128, H], bf16, tag="tp")
                    for fc in range(HC):
                        nc.tensor.transpose(
                            tp[:, fc * 128:(fc + 1) * 128],
                            Sp[:, fc, jc * 128:(jc + 1) * 128],
                            ident[:],
                        )
                    if jc % 2:
                        nc.scalar.activation(STl[:, jc, :], tp[:], ACT_COPY)
                    else:
                        nc.vector.tensor_copy(STl[:, jc, :], tp[:])
                ST_of[lvl] = STl

            def emit_sq(lvl):
                STl = ST_of[lvl]
                Sp = S_of[lvl - 1]
                Snew = spool.tile([128, HC, H], bf16, tag=f"S{lvl}")
                for mcc in range(HC):
                    ps = mmpool.tile([128, H], f32, tag="mm")
                    for jc in range(HC):
                        nc.tensor.matmul(
                            ps[:],
                            STl[:, jc, mcc * 128:(mcc + 1) * 128],
                            Sp[:, jc, :],
                            start=(jc == 0),
                            stop=(jc == HC - 1),
                        )
                    if mcc % 2:
                        nc.scalar.activation(Snew[:, mcc, :], ps[:], ACT_COPY)
                    else:
                        nc.vector.tensor_copy(Snew[:, mcc, :], ps[:])
                S_of[lvl] = Snew
                if lvl < NSQ:
                    emit_sq_transposes(lvl + 1)

            def emit_gmat(dst, lhs, rmat):
                """dst[:, gc, :] = sum_kc lhs-chunk^T @ rmat-chunk (bf16 out)."""
                for gc in range(ICH):
                    ps = mmpool.tile([128, H], f32, tag="mm")
                    for kc in range(HC):
                        nc.tensor.matmul(
                            ps[:],
                            lhs[:, kc, gc * 128:(gc + 1) * 128],
                            rmat[:, kc, :],
                            start=(kc == 0),
                            stop=(kc == HC - 1),
                        )
                    nc.vector.tensor_copy(dst[:, gc, :], ps[:])

            # G1 = Wxh^T A (layout like wxhT); G2 = G0 A^2; G3 = G1 A^2.
            # G2/G3 need S1, so they are emitted after sq(1).
            G1 = cpool.tile([128, ICH, H], bf16, tag="G1")
            G2 = cpool.tile([128, ICH, H], bf16, tag="G2")
            G3 = cpool.tile([128, ICH, H], bf16, tag="G3")

            # G2 = W' A^2 needs lhsT[p, m] = W'[gc*128+m, kc*128+p] -- the
            # TRANSPOSE of wxhT chunks.  Build wxhTT once via the dma xbar
            # (on the SP ring: the scalar ring's head will be blocked by the
            # sem-waiting squaring transposes); same for G1 -> G1T.
            wxhTT = cpool.tile([128, HC, IN], bf16, tag="wxhTT")
            G1T = cpool.tile([128, HC, IN], bf16, tag="G1T")

            def emit_wtt(dst, src_t):
                """dst[q, kc, ic*128+m] = src_t[m, ic, kc*128+q] via PE."""
                for kc in range(HC):
                    tp = trpool.tile([128, IN], bf16, tag="tp")
                    for ic in range(ICH):
                        nc.tensor.transpose(
                            tp[:, ic * 128:(ic + 1) * 128],
                            src_t[:, ic, kc * 128:(kc + 1) * 128],
                            ident[:],
                        )
                    if kc % 2:
                        nc.scalar.activation(dst[:, kc, :], tp[:], ACT_COPY)
                    else:
                        nc.vector.tensor_copy(dst[:, kc, :], tp[:])

            def emit_ggrp(dst, lhsTT, smat):
                """dst[:, gc, :] = (W_x A^2)[gc-chunk rows, :] with
                lhsTT[p, kc, gc*128+m] = W_x[gc*128+m, kc*128+p]."""
                for gc in range(ICH):
                    ps = mmpool.tile([128, H], f32, tag="mm")
                    for kc in range(HC):
                        nc.tensor.matmul(
                            ps[:],
                            lhsTT[:, kc, gc * 128:(gc + 1) * 128],
                            smat[:, kc, :],
                            start=(kc == 0),
                            stop=(kc == HC - 1),
                        )
                    nc.vector.tensor_copy(dst[:, gc, :], ps[:])

            # ---- schedule ------------------------------------------
            # xbar transposes for the G-matrix lhsTs (depend on wxhT only;
            # SP ring so they are not queued behind sem-waiting transposes)
            emit_wtt(wxhTT, wxhT)

            # sq1 first (serial chain), G's + projection as PE filler.
            emit_sq(1)  # S1 = A^2; also issues xbar transposes for sq2

            emit_gmat(G1, wnat, s0)  # G1 = W' A
            emit_wtt(G1T, G1)

            emit_ggrp(G2, wxhTT, S_of[1])   # G2 = W' A^2
            emit_ggrp(G3, G1T, S_of[1])     # G3 = (W'A) A^2

            # ---- projection fused with tree levels 0+1 (quad fold):
            # out_c = x_{4c+3} W' + x_{4c+2} G1 + x_{4c+1} G2 + x_{4c} G3
            #         + bias4,   giving the level-2 state directly.
            buf = lpool.tile([128, HC, COLS // 4], bf16, tag="L2")

            def emit_proj_half(hf):
                xh = xh_tiles[hf]
                mats = (wxhT, G1, G2, G3)
                offs = (3, 2, 1, 0)
                for mcc in range(HC):
                    ps = mmpool.tile([128, 512], f32, tag="mm")
                    first = True
                    for mi, mat in enumerate(mats):
                        for ic in range(ICH):
                            nc.tensor.matmul(
                                ps[:],
                                mat[:, ic, mcc * 128:(mcc + 1) * 128],
                                xh[:, ic, offs[mi]::4],
                                start=first,
                                stop=(mi == 3 and ic == ICH - 1),
                            )
                            first = False
                    nc.scalar.activation(
                        buf[:, mcc, hf * 512:(hf + 1) * 512],
                        ps[:],
                        ACT_IDENT,
                        bias=bias4[:, mcc:mcc + 1],
                    )

            # ---- tree level emission (levels 2..7)
            def emit_tree(lvl, buf):
                Sl = S_of[lvl]
                in_cols = COLS // (2 ** lvl)
                o_cols = in_cols // 2
                nbuf = lpool.tile([128, HC, o_cols], bf16, tag=f"L{lvl + 1}")
                if o_cols <= 64:
                    n = o_cols
                    ps = wpool.tile([128, HC, n], f32, tag="sm")
                    for mcc in range(HC):
                        for kc in range(HC):
                            nc.tensor.matmul(
                                ps[:, mcc, :],
                                Sl[:, kc, mcc * 128:(mcc + 1) * 128],
                                buf[:, kc, 0:2 * n:2],
                                start=(kc == 0),
                                stop=(kc == HC - 1),
                            )
                    nc.vector.tensor_add(
                        nbuf[:, :, :], ps[:], buf[:, :, 1:2 * n:2]
                    )
                    return nbuf
                nwin = (o_cols + 511) // 512
                for w in range(nwin):
                    n = min(512, o_cols - w * 512)
                    base = 1024 * w
                    for mcc in range(HC):
                        ps = mmpool.tile([128, n], f32, tag="mm")
                        for kc in range(HC):
                            nc.tensor.matmul(
                                ps[:],
                                Sl[:, kc, mcc * 128:(mcc + 1) * 128],
                                buf[:, kc, base:base + 2 * n:2],
                                start=(kc == 0),
                                stop=(kc == HC - 1),
                            )
                        nc.vector.tensor_add(
                            nbuf[:, mcc, w * 512:w * 512 + n],
                            ps[:],
                            buf[:, mcc, base + 1:base + 2 * n:2],
                        )
                return nbuf

            # squarings interleaved with projection groups (PE filler
            # between serial-chain steps)
            emit_sq(2)
            emit_proj_half(0)
            emit_sq(3)
            emit_proj_half(1)
            emit_sq(4)
            buf = emit_tree(2, buf)      # 1024 -> 512 cols
            emit_sq(5)
            buf = emit_tree(3, buf)      # 512 -> 256
            buf = emit_tree(4, buf)      # 256 -> 128
            emit_sq(6)
            buf = emit_tree(5, buf)      # 128 -> 64
            buf = emit_tree(6, buf)      # 64 -> 32
            emit_sq(7)
            buf = emit_tree(7, buf)      # 32 -> 16   (level-8 state)

            # ---- Horner tail over the 4 remaining chunks per batch row:
            # h_b = ((w0 Q + w1) Q + w2) Q + w3,  Q = A^256 = S7 . S7
            S7 = S_of[7]
            acc = None
            hout = None
            for j in (1, 2, 3):
                psA = wpool.tile([128, HC, BC], f32, tag="sm")
                for mcc in range(HC):
                    for kc in range(HC):
                        rhs = buf[:, kc, 0::4] if acc is None else acc[:, kc, :]
                        nc.tensor.matmul(
                            psA[:, mcc, :],
                            S7[:, kc, mcc * 128:(mcc + 1) * 128],
                            rhs,
                            start=(kc == 0),
                            stop=(kc == HC - 1),
                        )
                tmid = lpool.tile([128, HC, BC], bf16, tag=f"t{j}")
                nc.vector.tensor_copy(tmid[:, :, :], psA[:])
                psB = wpool.tile([128, HC, BC], f32, tag="sm")
                for mcc in range(HC):
                    for kc in range(HC):
                        nc.tensor.matmul(
                            psB[:, mcc, :],
                            S7[:, kc, mcc * 128:(mcc + 1) * 128],
                            tmid[:, kc, :],
                            start=(kc == 0),
                            stop=False,
                        )
                    nc.tensor.matmul(
                        psB[:, mcc, :],
                        ident[:],
                        buf[:, mcc, j::4],
                        start=False,
                        stop=True,
                    )
                if j < 3:
                    acc = lpool.tile([128, HC, BC], bf16, tag=f"acc{j}")
                    nc.vector.tensor_copy(acc[:, :, :], psB[:])
                else:
                    hout = lpool.tile([128, HC, BC], f32, tag="hout")
                    nc.vector.tensor_copy(hout[:, :, :], psB[:])

            nc.sync.dma_start(
                out_d.rearrange("p (c b) -> p c b", b=BC), hout[:, :, :]
            )

    nc.compile()
    return nc


def _get_nc():
    if "nc" not in _cache:
        _cache["nc"] = _build()
    return _cache["nc"]


def _in_maps(inputs):
    bf = ml_dtypes.bfloat16
    x = np.asarray(inputs["x"], dtype=np.float32)
    wxh = np.asarray(inputs["Wxh"], dtype=np.float32)
    bxh = np.asarray(inputs["bxh"], dtype=np.float32)
    whh = np.asarray(inputs["Whh"], dtype=np.float32)

    A = whh.T  # [H, H], row-vector convention
    wxhT = np.ascontiguousarray(
        wxh.reshape(H, ICH, 128).transpose(2, 1, 0).astype(bf)
    )  # [128, ICH, H] : W'[ic*128+p, f] with W' = Wxh^T
    wnat = np.ascontiguousarray(
        wxh.reshape(HC, 128, IN).transpose(1, 0, 2).astype(bf)
    )  # [128, HC, IN] : Wxh[kc*128+p, i]
    s0 = np.ascontiguousarray(
        A.reshape(HC, 128, H).transpose(1, 0, 2).astype(bf)
    )  # [128, HC, H] : A[kc*128+p, f]
    st1 = np.ascontiguousarray(
        whh.reshape(HC, 128, H).transpose(1, 0, 2).astype(bf)
    )  # [128, HC, H] : A^T[jc*128+p, f]
    b2 = bxh + bxh @ A
    b4 = b2 + b2 @ (A @ A)
    bias4 = np.ascontiguousarray(b4.reshape(HC, 128).T.astype(np.float32))
    ident = np.eye(128, dtype=np.float32).astype(bf)

    maps = []
    for c in range(NCORES):
        xc = x[c * BC:(c + 1) * BC].reshape(COLS, IN)
        xT = np.ascontiguousarray(
            xc.reshape(COLS, ICH, 128).transpose(2, 1, 0).astype(bf)
        )  # [128, ICH, COLS]
        maps.append(
            dict(
                xT=xT, wxhT=wxhT, wnat=wnat, s0=s0, st1=st1,
                bias4=bias4, ident=ident,
            )
        )
    return maps


def kernel(**inputs) -> np.ndarray:
    from concourse.bass_utils import run_bass_kernel_spmd

    res = run_bass_kernel_spmd(
        _get_nc(), _in_maps(inputs), list(range(NCORES))
    ).results
    return _assemble(res)


def _assemble(results) -> np.ndarray:
    outs = []
    for c in range(NCORES):
        o = np.asarray(results[c]["h_out"])      # [128, HC*BC] on-chip layout
        o = o.reshape(128, HC, BC).transpose(2, 1, 0).reshape(BC, H)
        outs.append(o)
    return np.concatenate(outs, axis=0).astype(np.float32)


# revision 40
# speedup vs baseline: 1.3302x; 1.1086x over previous
"""LinearRNN final-state kernel for 8 Trainium2 NeuronCores.

Reference computation:
    u_t = Wxh @ x_t + bxh            (input projection)
    h_t = u_t + Whh @ h_{t-1}        (recurrence over T=1024 steps)
    return h_T                        -> [B=32, H=512]

The recurrence is linear, so the final state is
    h_T = sum_t u_t @ A^(T-1-t),  A = Whh^T   (row-vector convention).
Tree fold: at level l adjacent sequence chunks pair up as
v' = v_odd + v_even @ A^(2^l).  All work is dense GEMMs; the only
sequential chain is the squaring chain A^(2^(l+1)) = (A^(2^l))^2.

Sharding: data-parallel over batch (B=32 -> 4 rows/core on 8 cores);
weights and the squaring chain are replicated.

This version:
  * all matmul operands are bf16 (1 cycle/row at any width on the PE);
    squarings 4-7 run in scaled fp8-e4m3 with DoubleRow perf mode
    (0.5 cycles/row, two K-chunks per instruction) -- safe because the
    terms touched by A^16.. powers are damped by ~0.9^16 and below,
  * every static transpose (x, Wxh, Whh) is done on the HOST as part of
    input marshalling -- the device receives matmul-ready layouts
    (remaining squaring-chain transposes use the PE; the DMA xbar
    corrupts under concurrent multi-core use),
  * the projection folds FOUR consecutive time steps at once using
    G_k = Wxh^T A^k for k=0..3, so the tree starts at level 2,
  * tree-level odd-column adds are folded into the PE via
    identity-matmul accumulation, leaving pure-copy epilogues split
    across ACT/DVE,
  * the last two tree levels run as a Horner chain with Q = A^256
    applied as two A^128 products.
"""

import numpy as np
import ml_dtypes

B, T, IN, H = 32, 1024, 256, 512
NCORES = 8
BC = B // NCORES          # 4 batch rows per core
COLS = BC * T             # 4096 sequence columns per core
HC = H // 128             # 4 hidden-dim chunks of 128
ICH = IN // 128           # 2 input-dim chunks
NG = 8                    # x groups of 512 columns
NSQ = 7                   # materialize A^2 .. A^128
WARMUPS = 16

_cache: dict = {}


def _build():
    import concourse.mybir as mybir
    from concourse import bacc
    from concourse.tile import TileContext

    f32 = mybir.dt.float32
    bf16 = mybir.dt.bfloat16
    fp8 = mybir.dt.float8e4
    DR = mybir.MatmulPerfMode.DoubleRow

    nc = bacc.Bacc(None)
    xT_d = nc.declare_dram_parameter("xT", [128, ICH, COLS], bf16, isOutput=False)
    wxhT_d = nc.declare_dram_parameter("wxhT", [128, ICH, H], bf16, isOutput=False)
    wnat_d = nc.declare_dram_parameter("wnat", [128, HC, IN], bf16, isOutput=False)
    s0_d = nc.declare_dram_parameter("s0", [128, HC, H], bf16, isOutput=False)
    st1_d = nc.declare_dram_parameter("st1", [128, HC, H], bf16, isOutput=False)
    bias4_d = nc.declare_dram_parameter("bias4", [128, HC], f32, isOutput=False)
    id_d = nc.declare_dram_parameter("ident", [128, 128], bf16, isOutput=False)
    out_d = nc.declare_dram_parameter("h_out", [128, HC * BC], f32, isOutput=True)

    ACT_COPY = mybir.ActivationFunctionType.Copy
    ACT_IDENT = mybir.ActivationFunctionType.Identity

    with TileContext(nc) as tc:
        with (
            tc.tile_pool(name="const", bufs=1) as cpool,
            tc.tile_pool(name="lvl", bufs=1) as lpool,
            tc.tile_pool(name="stream", bufs=3) as xpool,
            tc.tile_pool(name="mats", bufs=1) as spool,
            tc.tile_pool(name="stm", bufs=2) as stpool,
            tc.tile_pool(name="mm", bufs=5, space="PSUM") as mmpool,
            tc.tile_pool(name="tr", bufs=2, space="PSUM") as trpool,
            tc.tile_pool(name="sm", bufs=1, space="PSUM") as wpool,
        ):
            # ---- constant loads, spread across the three HWDGE rings.
            # The DMA device serializes transfers in arrival order, so the
            # squaring chain's operands (st1, s0) go first.
            st1 = cpool.tile([128, HC, H], bf16, tag="st1")
            s0 = cpool.tile([128, HC, H], bf16, tag="s0")
            wnat = cpool.tile([128, HC, IN], bf16, tag="wnat")
            for c in range(HC):
                nc.sync.dma_start(st1[:, c, :], st1_d[:, c, :])
                nc.scalar.dma_start(s0[:, c, :], s0_d[:, c, :])
            nc.sync.dma_start(wnat[:], wnat_d[:, :, :])
            wxhT = cpool.tile([128, ICH, H], bf16, tag="wxhT")
            nc.scalar.dma_start(wxhT[:], wxhT_d[:, :, :])
            ident = cpool.tile([128, 128], bf16, tag="ident")
            nc.sync.dma_start(ident[:], id_d[:, :])
            bias4 = cpool.tile([128, HC], f32, tag="bias4")
            nc.scalar.dma_start(bias4[:], bias4_d[:, :])

            # x halves (SP ring): [128, ICH, 2048] each
            xh_tiles = []
            for hf in range(2):
                xh = xpool.tile([128, ICH, 2048], bf16, tag="xh")
                nc.sync.dma_start(
                    xh[:], xT_d[:, :, hf * 2048:(hf + 1) * 2048]
                )
                xh_tiles.append(xh)

            # ---- PE warm-up on a memset tile (no DMA dependency): covers
            # the constant-DMA wait and starts the p-state ramp.
            warm_in = cpool.tile([128, 128], bf16, tag="warmin")
            nc.gpsimd.memset(warm_in[:], 0.0)
            for _ in range(WARMUPS):
                warm = mmpool.tile([128, 128], f32, tag="mm")
                nc.tensor.matmul(
                    warm[:], warm_in[:], warm_in[:], start=True, stop=True
                )

            # ---- squaring-chain pieces ------------------------------
            S_of = {0: s0}
            ST_of = {1: st1}
            # fp8 track: F_of[l] = S_l * 2^ALOG[l]  (unit-rms for e4m3)
            F_of = {}
            ALOG = {3: 6, 4: 7, 5: 9, 6: 14}

            def emit_sq_transposes(lvl):
                """ST_{lvl}[p, jc, mcc*128+m] = P[mcc*128+m, jc*128+p] via
                PE transposes (the DMA xbar corrupts under concurrent
                multi-core use).  For lvl >= 4 the source and result are the
                scaled fp8 track."""
                # Transposes always run in bf16 (fp8 PE-transpose has an
                # output-stride quirk); for lvl >= 4 the PSUM->SBUF copy
                # converts to the scaled fp8 track.
                Sp = S_of[lvl - 1]
                dt = fp8 if lvl >= 4 else bf16
                sc = 2.0 ** ALOG[lvl - 1] if lvl >= 4 else 1.0
                STl = stpool.tile([128, HC, H], dt, tag="ST")
                for jc in range(HC):
                    tp = trpool.tile([128, H], bf16, tag="tp")
                    for fc in range(HC):
                        nc.tensor.transpose(
                            tp[:, fc * 128:(fc + 1) * 128],
                            Sp[:, fc, jc * 128:(jc + 1) * 128],
                            ident[:],
                        )
                    dve = (jc % 2 == 0)
                    if not dve:
                        nc.scalar.activation(
                            STl[:, jc, :], tp[:], ACT_COPY, scale=sc
                        )
                    else:
                        if sc != 1.0:
                            nc.vector.tensor_scalar_mul(
                                STl[:, jc, :], tp[:], sc
                            )
                        else:
                            nc.vector.tensor_copy(STl[:, jc, :], tp[:])
                ST_of[lvl] = STl

            def emit_sq(lvl):
                """S_lvl = S_{lvl-1}^2.  Levels >= 4 run fp8 DoubleRow on the
                scaled track (psum = 2^pse * S_lvl); epilogues rescale.  The
                fp8 copies (F) gate the next chain step, so they are emitted
                before the bf16 exports (S)."""
                STl = ST_of[lvl]
                dr = lvl >= 4
                if dr:
                    Sp = F_of[lvl - 1]
                    pse = 2 * ALOG[lvl - 1]          # psum scale exponent
                else:
                    Sp = S_of[lvl - 1]
                    pse = 0
                Snew = spool.tile([128, HC, H], bf16, tag=f"S{lvl}")
                Fnew = None
                if lvl in ALOG:
                    Fnew = spool.tile([128, HC, H], fp8, tag=f"F{lvl}")
                    F_of[lvl] = Fnew
                pss = []
                for mcc in range(HC):
                    ps = mmpool.tile([128, H], f32, tag="mm")
                    pss.append(ps)
                    if dr:
                        for jp in range(2):
                            nc.tensor.matmul(
                                ps[:],
                                STl[:, 2 * jp:2 * jp + 2,
                                    mcc * 128:(mcc + 1) * 128],
                                Sp[:, 2 * jp:2 * jp + 2, :],
                                start=(jp == 0),
                                stop=(jp == 1),
                                perf_mode=DR,
                            )
                    else:
                        for jc in range(HC):
                            nc.tensor.matmul(
                                ps[:],
                                STl[:, jc, mcc * 128:(mcc + 1) * 128],
                                Sp[:, jc, :],
                                start=(jc == 0),
                                stop=(jc == HC - 1),
                            )
                if Fnew is not None:
                    fsc = 2.0 ** (ALOG[lvl] - pse)
                    for mcc in range(HC):
                        dve = (mcc % 2 == 1)
                        if dve:
                            nc.vector.tensor_scalar_mul(
                                Fnew[:, mcc, :], pss[mcc][:], fsc
                            )
                        else:
                            nc.scalar.activation(
                                Fnew[:, mcc, :], pss[mcc][:], ACT_COPY,
                                scale=fsc,
                            )
                for mcc in range(HC):
                    dve = (mcc % 2 == 0)
                    if not dve:
                        nc.scalar.activation(
                            Snew[:, mcc, :], pss[mcc][:], ACT_COPY,
                            scale=2.0 ** (-pse),
                        )
                    else:
                        if pse:
                            nc.vector.tensor_scalar_mul(
                                Snew[:, mcc, :], pss[mcc][:], 2.0 ** (-pse)
                            )
                        else:
                            nc.vector.tensor_copy(Snew[:, mcc, :], pss[mcc][:])
                S_of[lvl] = Snew

            def emit_gmat(dst, lhs, rmat):
                """dst[:, gc, :] = sum_kc lhs-chunk^T @ rmat-chunk (bf16 out)."""
                for gc in range(ICH):
                    ps = mmpool.tile([128, H], f32, tag="mm")
                    for kc in range(HC):
                        nc.tensor.matmul(
                            ps[:],
                            lhs[:, kc, gc * 128:(gc + 1) * 128],
                            rmat[:, kc, :],
                            start=(kc == 0),
                            stop=(kc == HC - 1),
                        )
                    nc.vector.tensor_copy(dst[:, gc, :], ps[:])

            # G1 = Wxh^T A (layout like wxhT); G2 = G0 A^2; G3 = G1 A^2.
            # G2/G3 need S1, so they are emitted after sq(1).
            G1 = cpool.tile([128, ICH, H], bf16, tag="G1")
            G2 = cpool.tile([128, ICH, H], bf16, tag="G2")
            G3 = cpool.tile([128, ICH, H], bf16, tag="G3")

            # G2 = W' A^2 reuses wnat as lhsT (wnat IS the chunk transpose
            # of W' = Wxh^T).  G3 = G1 A^2 needs G1T, built via PE transposes.
            G1T = cpool.tile([128, HC, IN], bf16, tag="G1T")

            def emit_wtt(dst, src_t):
                """dst[q, kc, ic*128+m] = src_t[m, ic, kc*128+q] via PE."""
                for kc in range(HC):
                    tp = trpool.tile([128, IN], bf16, tag="tp")
                    for ic in range(ICH):
                        nc.tensor.transpose(
                            tp[:, ic * 128:(ic + 1) * 128],
                            src_t[:, ic, kc * 128:(kc + 1) * 128],
                            ident[:],
                        )
                    if kc % 2:
                        nc.scalar.activation(dst[:, kc, :], tp[:], ACT_COPY)
                    else:
                        nc.vector.tensor_copy(dst[:, kc, :], tp[:])

            def emit_ggrp(dst, lhsTT, smat):
                """dst[:, gc, :] = (W_x A^2)[gc-chunk rows, :] with
                lhsTT[p, kc, gc*128+m] = W_x[gc*128+m, kc*128+p]."""
                for gc in range(ICH):
                    ps = mmpool.tile([128, H], f32, tag="mm")
                    for kc in range(HC):
                        nc.tensor.matmul(
                            ps[:],
                            lhsTT[:, kc, gc * 128:(gc + 1) * 128],
                            smat[:, kc, :],
                            start=(kc == 0),
                            stop=(kc == HC - 1),
                        )
                    nc.vector.tensor_copy(dst[:, gc, :], ps[:])

            # ---- schedule ------------------------------------------
            # sq1 first (serial chain), G's + projection as PE filler.
            emit_sq(1)                # S1 = A^2
            emit_gmat(G1, wnat, s0)   # G1 = W' A   (covers S1 epilogue)
            emit_sq_transposes(2)     # ST2 <- S1
            emit_wtt(G1T, G1)

            emit_ggrp(G2, wnat, S_of[1])    # G2 = W' A^2
            emit_ggrp(G3, G1T, S_of[1])     # G3 = (W'A) A^2

            # ---- projection fused with tree levels 0+1 (quad fold):
            # out_c = x_{4c+3} W' + x_{4c+2} G1 + x_{4c+1} G2 + x_{4c} G3
            #         + bias4,   giving the level-2 state directly.
            buf = lpool.tile([128, HC, COLS // 4], bf16, tag="L2")

            def emit_proj_half(hf, mccs=tuple(range(HC))):
                xh = xh_tiles[hf]
                mats = (wxhT, G1, G2, G3)
                offs = (3, 2, 1, 0)
                for mcc in mccs:
                    ps = mmpool.tile([128, 512], f32, tag="mm")
                    first = True
                    for mi, mat in enumerate(mats):
                        for ic in range(ICH):
                            nc.tensor.matmul(
                                ps[:],
                                mat[:, ic, mcc * 128:(mcc + 1) * 128],
                                xh[:, ic, offs[mi]::4],
                                start=first,
                                stop=(mi == 3 and ic == ICH - 1),
                            )
                            first = False
                    nc.scalar.activation(
                        buf[:, mcc, hf * 512:(hf + 1) * 512],
                        ps[:],
                        ACT_IDENT,
                        bias=bias4[:, mcc:mcc + 1],
                    )

            # ---- tree level emission (levels 2..7)
            def emit_tree(lvl, buf, mccs=tuple(range(HC)), nbuf=None):
                Sl = S_of[lvl]
                in_cols = COLS // (2 ** lvl)
                o_cols = in_cols // 2
                if nbuf is None:
                    nbuf = lpool.tile(
                        [128, HC, o_cols], bf16, tag=f"L{lvl + 1}"
                    )
                if o_cols <= 64:
                    n = o_cols
                    ps = wpool.tile([128, HC, n], f32, tag="sm")
                    for mcc in range(HC):
                        for kc in range(HC):
                            nc.tensor.matmul(
                                ps[:, mcc, :],
                                Sl[:, kc, mcc * 128:(mcc + 1) * 128],
                                buf[:, kc, 0:2 * n:2],
                                start=(kc == 0),
                                stop=(kc == HC - 1),
                            )
                    nc.vector.tensor_add(
                        nbuf[:, :, :], ps[:], buf[:, :, 1:2 * n:2]
                    )
                    return nbuf
                nwin = (o_cols + 511) // 512
                for w in range(nwin):
                    n = min(512, o_cols - w * 512)
                    base = 1024 * w
                    for mcc in mccs:
                        ps = mmpool.tile([128, n], f32, tag="mm")
                        for kc in range(HC):
                            nc.tensor.matmul(
                                ps[:],
                                Sl[:, kc, mcc * 128:(mcc + 1) * 128],
                                buf[:, kc, base:base + 2 * n:2],
                                start=(kc == 0),
                                stop=False,
                            )
                        # fold the odd-column add into the PE so the
                        # epilogue is a pure copy (splittable DVE/ACT)
                        nc.tensor.matmul(
                            ps[:],
                            ident[:],
                            buf[:, mcc, base + 1:base + 2 * n:2],
                            start=False,
                            stop=True,
                        )
                        if mcc % 2:
                            nc.scalar.activation(
                                nbuf[:, mcc, w * 512:w * 512 + n],
                                ps[:], ACT_COPY,
                            )
                        else:
                            nc.vector.tensor_copy(
                                nbuf[:, mcc, w * 512:w * 512 + n], ps[:]
                            )
                return nbuf

            # squarings interleaved with projection groups (PE filler
            # between serial-chain steps)
            emit_sq(2)
            emit_proj_half(0, (0,))
            emit_sq_transposes(3)
            emit_proj_half(0, (1, 2, 3))
            emit_sq(3)
            emit_proj_half(1, (0,))
            emit_sq_transposes(4)
            emit_proj_half(1, (1,))
            emit_sq(4)
            emit_proj_half(1, (2,))
            emit_sq_transposes(5)
            emit_proj_half(1, (3,))
            emit_sq(5)
            buf3 = emit_tree(2, buf, mccs=(0,))        # 1024 -> 512
            emit_sq_transposes(6)
            emit_tree(2, buf, mccs=(1, 2, 3), nbuf=buf3)
            emit_sq(6)
            buf4 = emit_tree(3, buf3, mccs=(0, 1))     # 512 -> 256
            emit_sq_transposes(7)
            emit_tree(3, buf3, mccs=(2, 3), nbuf=buf4)
            emit_sq(7)
            buf5 = emit_tree(4, buf4)                  # 256 -> 128
            buf6 = emit_tree(5, buf5)                  # 128 -> 64
            buf7 = emit_tree(6, buf6)                  # 64 -> 32
            buf = emit_tree(7, buf7)                   # 32 -> 16

            # ---- Horner tail over the 4 remaining chunks per batch row:
            # h_b = ((w0 Q + w1) Q + w2) Q + w3,  Q = A^256 = S7 . S7
            S7 = S_of[7]
            acc = None
            hout = None
            for j in (1, 2, 3):
                psA = wpool.tile([128, HC, BC], f32, tag="sm")
                for mcc in range(HC):
                    for kc in range(HC):
                        rhs = buf[:, kc, 0::4] if acc is None else acc[:, kc, :]
                        nc.tensor.matmul(
                            psA[:, mcc, :],
                            S7[:, kc, mcc * 128:(mcc + 1) * 128],
                            rhs,
                            start=(kc == 0),
                            stop=(kc == HC - 1),
                        )
                tmid = lpool.tile([128, HC, BC], bf16, tag=f"t{j}")
                nc.vector.tensor_copy(tmid[:, :, :], psA[:])
                psB = wpool.tile([128, HC, BC], f32, tag="sm")
                for mcc in range(HC):
                    for kc in range(HC):
                        nc.tensor.matmul(
                            psB[:, mcc, :],
                            S7[:, kc, mcc * 128:(mcc + 1) * 128],
                            tmid[:, kc, :],
                            start=(kc == 0),
                            stop=False,
                        )
                    nc.tensor.matmul(
                        psB[:, mcc, :],
                        ident[:],
                        buf[:, mcc, j::4],
                        start=False,
                        stop=True,
                    )
                if j < 3:
                    acc = lpool.tile([128, HC, BC], bf16, tag=f"acc{j}")
                    nc.vector.tensor_copy(acc[:, :, :], psB[:])
                else:
                    hout = lpool.tile([128, HC, BC], f32, tag="hout")
                    nc.vector.tensor_copy(hout[:, :, :], psB[:])
                    nc.sync.dma_start(
                        out_d.rearrange("p (c b) -> p c b", b=BC),
                        hout[:, :, :],
                    )

    nc.compile()
    return nc


def _get_nc():
    if "nc" not in _cache:
        _cache["nc"] = _build()
    return _cache["nc"]


def _in_maps(inputs):
    bf = ml_dtypes.bfloat16
    x = np.asarray(inputs["x"], dtype=np.float32)
    wxh = np.asarray(inputs["Wxh"], dtype=np.float32)
    bxh = np.asarray(inputs["bxh"], dtype=np.float32)
    whh = np.asarray(inputs["Whh"], dtype=np.float32)

    A = whh.T  # [H, H], row-vector convention
    wxhT = np.ascontiguousarray(
        wxh.reshape(H, ICH, 128).transpose(2, 1, 0).astype(bf)
    )  # [128, ICH, H] : W'[ic*128+p, f] with W' = Wxh^T
    wnat = np.ascontiguousarray(
        wxh.reshape(HC, 128, IN).transpose(1, 0, 2).astype(bf)
    )  # [128, HC, IN] : Wxh[kc*128+p, i]
    s0 = np.ascontiguousarray(
        A.reshape(HC, 128, H).transpose(1, 0, 2).astype(bf)
    )  # [128, HC, H] : A[kc*128+p, f]
    st1 = np.ascontiguousarray(
        whh.reshape(HC, 128, H).transpose(1, 0, 2).astype(bf)
    )  # [128, HC, H] : A^T[jc*128+p, f]
    b2 = bxh + bxh @ A
    b4 = b2 + b2 @ (A @ A)
    bias4 = np.ascontiguousarray(b4.reshape(HC, 128).T.astype(np.float32))
    ident = np.eye(128, dtype=np.float32).astype(bf)

    maps = []
    for c in range(NCORES):
        xc = x[c * BC:(c + 1) * BC].reshape(COLS, IN)
        xT = np.ascontiguousarray(
            xc.reshape(COLS, ICH, 128).transpose(2, 1, 0).astype(bf)
        )  # [128, ICH, COLS]
        maps.append(
            dict(
                xT=xT, wxhT=wxhT, wnat=wnat, s0=s0, st1=st1,
                bias4=bias4, ident=ident,
            )
        )
    return maps


def kernel(**inputs) -> np.ndarray:
    from concourse.bass_utils import run_bass_kernel_spmd

    res = run_bass_kernel_spmd(
        _get_nc(), _in_maps(inputs), list(range(NCORES))
    ).results
    return _assemble(res)


def _assemble(results) -> np.ndarray:
    outs = []
    for c in range(NCORES):
        o = np.asarray(results[c]["h_out"])      # [128, HC*BC] on-chip layout
        o = o.reshape(128, HC, BC).transpose(2, 1, 0).reshape(BC, H)
        outs.append(o)
    return np.concatenate(outs, axis=0).astype(np.float32)
